# revision 24
# baseline (speedup 1.0000x reference)
"""LocalRCT sparse-attention kernel for 8 Trainium2 NeuronCores.

Full inputs in, full output out. Sharding: core = (batch b = core//2,
row-half h = core%2). Each core processes feature[b, :, 256h:256h+256, :]
(8 strip-rows of 16 tiles = 128 tiles of 32x32 px) and computes the two
tiny conv blocks on its p_low slice on-device.
"""

import os
import sys

sys.path.insert(0, "/opt/trn_rl_repo")

import numpy as np
import ml_dtypes

import concourse.bacc as bacc
import concourse.bass as bass
import concourse.tile as tile
from concourse import mybir
from concourse.bass_utils import run_bass_kernel_spmd

F32 = mybir.dt.float32
BF16 = mybir.dt.bfloat16
AF = mybir.ActivationFunctionType

RF, NLF, MG, FUSION = 64, 16, 16, 64  # feat ch, nlf, mesh grid, fusion ch
TS = 32          # tile spatial size (px)
NROW = 8         # strip rows per core
NPAIR = 8        # tile pairs per strip
NPAIRS = NROW * NPAIR  # 64 pairs/core


def build_program():
    nc = bacc.Bacc("TRN2", target_bir_lowering=False, debug=False)

    feat = nc.dram_tensor("feat", [64, 256, 512], BF16, kind="ExternalInput")
    pshard = nc.dram_tensor("pshard", [64, 13, 21], BF16, kind="ExternalInput")
    w1rT = nc.dram_tensor("w1rT", [64, 9, 64], BF16, kind="ExternalInput")
    b1r = nc.dram_tensor("b1r", [64, 1], F32, kind="ExternalInput")
    w2rT = nc.dram_tensor("w2rT", [64, 9, 1024], BF16, kind="ExternalInput")
    b2r = nc.dram_tensor("b2r", [128, 8], F32, kind="ExternalInput")
    w1tT = nc.dram_tensor("w1tT", [64, 9, 64], BF16, kind="ExternalInput")
    b1t = nc.dram_tensor("b1t", [64, 1], F32, kind="ExternalInput")
    w2tT = nc.dram_tensor("w2tT", [64, 9, 48], BF16, kind="ExternalInput")
    b2t = nc.dram_tensor("b2t", [48, 1], F32, kind="ExternalInput")
    ones64 = nc.dram_tensor("ones64", [64, 64], BF16, kind="ExternalInput")
    hmask = nc.dram_tensor("hmask", [64, 11, 19], BF16, kind="ExternalInput")
    yshard = nc.dram_tensor("yshard", [3, 256, 512], F32, kind="ExternalOutput")

    with tile.TileContext(nc) as tc:
        with (
            tc.tile_pool(name="singles", bufs=1) as singles,
            tc.tile_pool(name="convout", bufs=2) as convout,
            tc.tile_pool(name="strips", bufs=3) as strips,
            tc.tile_pool(name="exps", bufs=6) as exps,
            tc.tile_pool(name="work", bufs=3) as work,
            tc.tile_pool(name="t2p", bufs=6) as t2p,
            tc.tile_pool(name="qkp", bufs=2, space="PSUM") as qkp,
            tc.tile_pool(name="avp", bufs=2, space="PSUM") as avp,
            tc.tile_pool(name="dram", bufs=1, space="DRAM") as dram,
        ):
            # ---------------- setup: load weights & p_low ----------------
            w1rT_s = singles.tile([64, 9, 64], BF16)
            w2rT_s = singles.tile([64, 9, 1024], BF16)
            w1tT_s = singles.tile([64, 9, 64], BF16)
            w2tT_s = singles.tile([64, 9, 48], BF16)
            b1r_s = singles.tile([64, 1], F32)
            b2r_s = singles.tile([128, 8], F32)
            b1t_s = singles.tile([64, 1], F32)
            b2t_s = singles.tile([48, 1], F32)
            p0 = singles.tile([64, 13, 21], BF16)
            hmask_s = singles.tile([64, 11, 19], BF16)
            setup_engs = [nc.sync, nc.scalar]
            for si, (dst, src) in enumerate([
                (p0, pshard), (w1tT_s, w1tT), (w2tT_s, w2tT), (b1t_s, b1t),
                (b2t_s, b2t), (hmask_s, hmask), (w1rT_s, w1rT),
                (w2rT_s, w2rT), (b1r_s, b1r), (b2r_s, b2r),
            ]):
                setup_engs[si % 2].dma_start(out=dst[:], in_=src[:])

            rl_d = dram.tile([1024, 9, 17], BF16)
            tl_d = dram.tile([48, 9, 17], BF16)

            def load_strip(ti):
                s = strips.tile([128, 32, 8, 32], BF16, tag="strip")
                nc.sync.dma_start(out=s[0:64],
                                  in_=feat[:, 32 * ti:32 * (ti + 1), 0:256])
                nc.sync.dma_start(out=s[64:128],
                                  in_=feat[:, 32 * ti:32 * (ti + 1), 256:512])
                return s

            # prefetch every strip before the gather DMAs enter the SP
            # ring FIFO: feature traffic overlaps the conv blocks and the
            # main loop never waits on a strip load
            prefetched = {ti: load_strip(ti) for ti in range(2)}

            # ---------------- conv blocks (valid convs on padded slice) --
            # conv1: [64,13,21] -> [64,11,19], silu; conv2: -> [C,9,17]
            def rct_block(w1T_s, b1_s, w2T_s, b2_s, cout, out_d):
                c1 = avp.tile([64, 11, 19], F32, tag="av_t")
                for k in range(9):
                    dy, dx = k // 3, k % 3
                    nc.tensor.matmul(
                        out=c1[:],
                        lhsT=w1T_s[:, k, :],
                        rhs=p0[:, dy:dy + 11, dx:dx + 19],
                        start=(k == 0), stop=(k == 8),
                    )
                h1 = singles.tile([64, 11, 19], BF16)
                nc.scalar.activation(out=h1[:], in_=c1[:], func=AF.Silu,
                                     bias=b1_s[:], scale=1.0)
                # zero ring outside the global 17x17 grid (reference zero-pads
                # conv2 input); mask is core-dependent data
                nc.vector.tensor_mul(h1[:], h1[:], hmask_s[:])
                for oc in range(cout // 128 if cout >= 128 else 1):
                    mw = min(128, cout)
                    c2 = avp.tile([mw, 9, 17], F32, tag="av_t")
                    for k in range(9):
                        dy, dx = k // 3, k % 3
                        nc.tensor.matmul(
                            out=c2[:],
                            lhsT=w2T_s[:, k, oc * 128:oc * 128 + mw],
                            rhs=h1[:, dy:dy + 9, dx:dx + 17],
                            start=(k == 0), stop=(k == 8),
                        )
                    o2 = convout.tile([mw, 9, 17], BF16, tag="o2")
                    if mw == 128:
                        bias_ap = b2_s[:, oc:oc + 1]
                    else:
                        bias_ap = b2_s[:]
                    nc.scalar.activation(out=o2[:], in_=c2[:], func=AF.Identity,
                                         bias=bias_ap, scale=1.0)
                    nc.scalar.dma_start(out=out_d[oc * 128:oc * 128 + mw],
                                        in_=o2[:])

            rct_block(w1tT_s, b1t_s, w2tT_s, b2t_s, 48, tl_d)
            rct_block(w1rT_s, b1r_s, w2rT_s, b2r_s, 1024, rl_d)

            # ---------------- gather rk / tv from scratch ----------------
            # Scratch channel order is l-major: rl ch' = l*64 + r (via host
            # weight-col permutation), tl ch' = l*3 + ch.
            # rk_blk[64a+r, k*64 + pair] = r_l[16r+l, ti+di, jjp+8a+dj]
            #   with k = 16*(2di+dj) + l, pair = ti*8 + jjp (tiles j, j+8).
            # tv_blk[64a+k', s*64 + pair] = t_l[ch*16+l, ...], s = 4a+ch;
            #   s = 4a+3 is a ones column (softmax denominator).
            rk_blk = singles.tile([128, 4096], BF16)
            tv_blk = singles.tile([128, 512], BF16)
            nc.vector.memset(tv_blk[:], 0.0)

            rl5 = rl_d[:].rearrange("(l r) y x -> l r y x", l=16, r=64)
            tl5 = tl_d[:].rearrange("(l ch) y x -> l ch y x", l=16, ch=3)
            rkv = rk_blk.rearrange("p (c l pr) -> p c l pr", c=4, l=16, pr=64)
            tvv = tv_blk.rearrange("p (s ti jjp) -> p s ti jjp", s=8, ti=8, jjp=8)
            # tv first (conv-t finishes early), then rk ti-major so
            # strip 0's keys are ready first; alternate rings so neither
            # sequencer serializes the whole gather
            gi = 0
            for a in range(2):
                for di in range(2):
                    for dj in range(2):
                        c4 = 2 * di + dj
                        for ch in range(3):
                            src = tl5[:, ch, di:di + 8,
                                      8 * a + dj:8 * a + dj + 8]
                            setup_engs[gi % 2].dma_start(
                                out=tvv[64 * a + 16 * c4:64 * a + 16 * (c4 + 1),
                                        4 * a + ch, :, :],
                                in_=src)
                            gi += 1
                # ones column (softmax denominator accumulator)
                setup_engs[gi % 2].dma_start(
                    out=tvv[64 * a:64 * (a + 1), 4 * a + 3].rearrange(
                        "p ti jjp -> p (ti jjp)"),
                    in_=ones64[:])
                gi += 1
            for ti in range(8):
                for a in range(2):
                    for di in range(2):
                        for dj in range(2):
                            c4 = 2 * di + dj
                            src = rl5[:, :, ti + di, 8 * a + dj:8 * a + dj + 8]
                            setup_engs[gi % 2].dma_start(
                                out=rkv[64 * a:64 * (a + 1), c4, :,
                                        8 * ti:8 * (ti + 1)],
                                in_=src.transpose([1, 0, 2]))
                            gi += 1

            rkk = rk_blk.rearrange("p (k pr) -> p k pr", k=64, pr=64)
            tvs = tv_blk.rearrange("p (s pr) -> p s pr", s=8, pr=64)

            # ---------------- main loop ----------------
            # KREPEAT>1 re-runs the hot loop in-NEFF (benchmarking only:
            # amplifies kernel time above the per-launch overhead floor).
            repeat = int(os.environ.get("KREPEAT", "1"))
            for ti in [t for _ in range(repeat) for t in range(NROW)]:
                strip = (prefetched.pop(ti) if ti in prefetched
                         else load_strip(ti))

                for g4 in range(2):
                    av_t = avp.tile([128, 1024], F32, tag="av_t")
                    for u in range(4):
                        jjp = 4 * g4 + u
                        pair = ti * 8 + jjp
                        w_, uu = u // 2, u % 2
                        qk_t = qkp.tile([128, 1024], F32, tag="qk_t")
                        lhsA = rkk[0:64, :, pair]
                        lhsB = rkk[64:128, :, pair]
                        for half, lhsH in ((0, lhsA), (64, lhsB)):
                            for v in range(2):
                                rhsH = strip[half:half + 64,
                                             16 * v:16 * (v + 1), jjp, :]
                                nc.tensor.matmul(
                                    out=qk_t[half:half + 64,
                                             512 * v:512 * (v + 1)],
                                    lhsT=lhsH, rhs=rhsH)
                        exp_t = exps.tile([128, 1024], BF16, tag="exp_t")
                        nc.scalar.activation(out=exp_t[:], in_=qk_t[:],
                                             func=AF.Exp, scale=0.125)
                        # AV+sum: out rows (4a+s) = (chA0..2,sumA,chB0..2,sumB)
                        lhsV = tvs[:, :, pair]
                        for v in range(2):
                            po = 32 * (2 * uu + v)
                            nc.tensor.matmul(
                                out=av_t[po:po + 8, 512 * w_:512 * (w_ + 1)],
                                lhsT=lhsV,
                                rhs=exp_t[:, 512 * v:512 * (v + 1)],
                                tile_position=(0, po))

                    # normalize 2 banks (4 pairs x 2 chunks quadrants).
                    # muls scatter into mt2 free=(w,s | 2i+a); the second
                    # 32x32 transpose then lands (v,2i,a) on partitions so
                    # the store balances to a 3-dim DMA per (uu,w):
                    # t2b[64uu + 32v+2i+a, 128w+32s+q] -> yshard[s, y, x]
                    tt = work.tile([128, 1024], F32, tag="tt")
                    nc.vector.transpose(out=tt[:], in_=av_t[:])
                    t5 = tt.rearrange("p (w i r) -> p w i r", w=2, i=16,
                                      r=32)[:, :, :, 0:8].rearrange(
                        "p w i (aa s) -> p w i aa s", aa=2, s=4)
                    rc = work.tile([128, 2, 16, 2], F32, tag="rc")
                    nc.vector.reciprocal(out=rc[:], in_=t5[:, :, :, :, 3])
                    mt2 = work.tile([128, 256], F32, tag="mt")
                    m2v = mt2.rearrange("p (w s iq) -> p w s iq", w=2, s=4,
                                        iq=32).rearrange(
                        "p w s (i aa) -> p w s i aa", i=16, aa=2)
                    for s in range(4):
                        nc.vector.tensor_mul(m2v[:, :, s], t5[:, :, :, :, s],
                                             rc[:])
                    t2b = t2p.tile([128, 256], F32, tag="t2")
                    nc.vector.transpose(out=t2b[:], in_=mt2[:])
                    Yv = yshard[:, 32 * ti:32 * (ti + 1), :].rearrange(
                        "c y x -> c (y x)").rearrange(
                        "c (via rem) -> c via rem", via=64, rem=256)
                    for uu in range(2):
                        for w in range(2):
                            srcap = t2b[64 * uu:64 * (uu + 1),
                                        128 * w:128 * (w + 1)].rearrange(
                                "p (s q) -> p s q", s=4, q=32)[:, 0:3, :]
                            b = 128 * g4 + 64 * w + 32 * uu
                            out_engs = [nc.sync, nc.scalar]
                            out_engs[(w + uu) % 2].dma_start(
                                out=Yv[:, :, b:b + 32].transpose([1, 0, 2]),
                                in_=srcap)
    nc.compile()
    return nc


_PROGRAM_CACHE = {}


def _get_program():
    if "nc" not in _PROGRAM_CACHE:
        _PROGRAM_CACHE["nc"] = build_program()
    return _PROGRAM_CACHE["nc"]


def _prep_inputs(feature, p_low, r_w1, r_b1, r_gamma, r_beta, r_mean, r_var,
                 r_w2, r_b2, t_w1, t_b1, t_gamma, t_beta, t_mean, t_var,
                 t_w2, t_b2):
    f32 = np.float32

    def fold(w1, b1, g, be, m, v):
        s = (g / np.sqrt(v + f32(1e-5))).astype(f32)
        w1f = (w1 * s[:, None, None, None]).astype(f32)
        b1f = ((b1 - m) * s + be).astype(f32)
        return w1f, b1f

    def wT(w):  # [O,C,3,3] -> [C, 9, O]
        return np.ascontiguousarray(w.transpose(1, 2, 3, 0).reshape(
            w.shape[1], 9, w.shape[0])).astype(ml_dtypes.bfloat16)

    w1rf, b1rf = fold(r_w1, r_b1, r_gamma, r_beta, r_mean, r_var)
    w1tf, b1tf = fold(t_w1, t_b1, t_gamma, t_beta, t_mean, t_var)

    # conv2 weight cols permuted l-major: rl col o' = l*64 + r for
    # o = r*16 + l; tl col o' = l*3 + ch for o = ch*16 + l.
    w2r_lm = np.ascontiguousarray(
        r_w2.reshape(64, 16, 64, 3, 3).transpose(2, 3, 4, 1, 0).reshape(
            64, 9, 1024)).astype(f32)
    b2r_lm = np.ascontiguousarray(
        r_b2.reshape(64, 16).T.reshape(1024).reshape(8, 128).T).astype(f32)
    w2t_lm = np.ascontiguousarray(
        t_w2.reshape(3, 16, 64, 3, 3).transpose(2, 3, 4, 1, 0).reshape(
            64, 9, 48)).astype(f32)
    b2t_lm = np.ascontiguousarray(
        t_b2.reshape(3, 16).T.reshape(48, 1)).astype(f32)

    common = {
        "w1rT": wT(w1rf), "b1r": b1rf.reshape(64, 1),
        "w2rT": w2r_lm.astype(ml_dtypes.bfloat16), "b2r": b2r_lm,
        "w1tT": wT(w1tf), "b1t": b1tf.reshape(64, 1),
        "w2tT": w2t_lm.astype(ml_dtypes.bfloat16), "b2t": b2t_lm,
        "ones64": np.ones((64, 64), ml_dtypes.bfloat16),
    }

    # p_low padded: [B, 64, 17, 17] -> per (b,h): [64, 13, 21]
    ppad = np.zeros((p_low.shape[0], 64, 17 + 8, 21), f32)
    ppad[:, :, 2:19, 2:19] = p_low
    in_maps = []
    for core in range(8):
        b, h = core // 2, core % 2
        m = dict(common)
        m["feat"] = np.ascontiguousarray(feature[b, :, 256 * h:256 * (h + 1), :]
                                         ).astype(ml_dtypes.bfloat16)
        m["pshard"] = np.ascontiguousarray(ppad[b, :, 8 * h:8 * h + 13, :]).astype(ml_dtypes.bfloat16)
        mk = np.zeros((11, 19), f32)
        for i in range(11):
            for j in range(19):
                gr, gc = 8 * h - 1 + i, j - 1
                if 0 <= gr <= 16 and 0 <= gc <= 16:
                    mk[i, j] = 1.0
        m["hmask"] = np.ascontiguousarray(
            np.broadcast_to(mk, (64, 11, 19))).astype(ml_dtypes.bfloat16)
        in_maps.append(m)
    return in_maps


def run(inputs, trace=False, tmpdir=None):
    nc = _get_program()
    in_maps = _prep_inputs(**{k: np.asarray(v) for k, v in inputs.items()})
    res = run_bass_kernel_spmd(nc, in_maps, core_ids=list(range(8)),
                               trace=trace, tmpdir=tmpdir)
    Y = np.zeros((4, 3, 512, 512), np.float32)
    for core in range(8):
        b, h = core // 2, core % 2
        Y[b, :, 256 * h:256 * (h + 1), :] = res.results[core]["yshard"]
    return Y, res


def kernel(**inputs):
    return run(inputs)[0]



# revision 29
# speedup vs baseline: 1.2783x; 1.2783x over previous
"""LocalRCT sparse-attention kernel for 8 Trainium2 NeuronCores.

Full inputs in, full output out. Sharding: core = (batch b = core//2,
row-half h = core%2). Each core processes feature[b, :, 256h:256h+256, :]
(8 strip-rows of 16 tiles = 128 tiles of 32x32 px) and computes the two
tiny conv blocks on its p_low slice on-device.
"""

import os
import sys

sys.path.insert(0, "/opt/trn_rl_repo")

import numpy as np
import ml_dtypes

import concourse.bacc as bacc
import concourse.bass as bass
import concourse.tile as tile
from concourse import mybir
from concourse.bass_utils import run_bass_kernel_spmd

F32 = mybir.dt.float32
BF16 = mybir.dt.bfloat16
AF = mybir.ActivationFunctionType

RF, NLF, MG, FUSION = 64, 16, 16, 64  # feat ch, nlf, mesh grid, fusion ch
TS = 32          # tile spatial size (px)
NROW = 8         # strip rows per core
NPAIR = 8        # tile pairs per strip
NPAIRS = NROW * NPAIR  # 64 pairs/core


def build_program():
    nc = bacc.Bacc("TRN2", target_bir_lowering=False, debug=False)

    feat = nc.dram_tensor("feat", [64, 256, 512], BF16, kind="ExternalInput")
    pshard = nc.dram_tensor("pshard", [64, 13, 21], BF16, kind="ExternalInput")
    w1rT = nc.dram_tensor("w1rT", [64, 9, 64], BF16, kind="ExternalInput")
    b1r = nc.dram_tensor("b1r", [64, 1], F32, kind="ExternalInput")
    w2rT = nc.dram_tensor("w2rT", [64, 9, 1024], BF16, kind="ExternalInput")
    b2r = nc.dram_tensor("b2r", [128, 8], F32, kind="ExternalInput")
    w1tT = nc.dram_tensor("w1tT", [64, 9, 64], BF16, kind="ExternalInput")
    b1t = nc.dram_tensor("b1t", [64, 1], F32, kind="ExternalInput")
    w2tT = nc.dram_tensor("w2tT", [64, 9, 48], BF16, kind="ExternalInput")
    b2t = nc.dram_tensor("b2t", [48, 1], F32, kind="ExternalInput")
    ones64 = nc.dram_tensor("ones64", [64, 64], BF16, kind="ExternalInput")
    hmask = nc.dram_tensor("hmask", [64, 11, 19], BF16, kind="ExternalInput")
    yshard = nc.dram_tensor("yshard", [3, 256, 512], F32, kind="ExternalOutput")

    with tile.TileContext(nc) as tc:
        with (
            tc.tile_pool(name="singles", bufs=1) as singles,
            tc.tile_pool(name="convout", bufs=2) as convout,
            tc.tile_pool(name="strips", bufs=3) as strips,
            tc.tile_pool(name="exps", bufs=6) as exps,
            tc.tile_pool(name="work", bufs=3) as work,
            tc.tile_pool(name="t2p", bufs=6) as t2p,
            tc.tile_pool(name="qkp", bufs=2, space="PSUM") as qkp,
            tc.tile_pool(name="avp", bufs=2, space="PSUM") as avp,
            tc.tile_pool(name="dram", bufs=1, space="DRAM") as dram,
        ):
            # ---------------- setup: load weights & p_low ----------------
            w1rT_s = singles.tile([64, 9, 64], BF16)
            w2rT_s = singles.tile([64, 9, 1024], BF16)
            w1tT_s = singles.tile([64, 9, 64], BF16)
            w2tT_s = singles.tile([64, 9, 48], BF16)
            b1r_s = singles.tile([64, 1], F32)
            b2r_s = singles.tile([128, 8], F32)
            b1t_s = singles.tile([64, 1], F32)
            b2t_s = singles.tile([48, 1], F32)
            p0 = singles.tile([64, 13, 21], BF16)
            hmask_s = singles.tile([64, 11, 19], BF16)
            setup_engs = [nc.sync, nc.scalar]
            for si, (dst, src) in enumerate([
                (p0, pshard), (w1tT_s, w1tT), (w2tT_s, w2tT), (b1t_s, b1t),
                (b2t_s, b2t), (hmask_s, hmask), (w1rT_s, w1rT),
                (w2rT_s, w2rT), (b1r_s, b1r), (b2r_s, b2r),
            ]):
                setup_engs[si % 2].dma_start(out=dst[:], in_=src[:])

            rl_d = dram.tile([1024, 9, 17], BF16)
            tl_d = dram.tile([48, 9, 17], BF16)

            def load_strip(ti):
                s = strips.tile([128, 32, 8, 32], BF16, tag="strip")
                nc.sync.dma_start(out=s[0:64],
                                  in_=feat[:, 32 * ti:32 * (ti + 1), 0:256])
                nc.sync.dma_start(out=s[64:128],
                                  in_=feat[:, 32 * ti:32 * (ti + 1), 256:512])
                return s

            # prefetch every strip before the gather DMAs enter the SP
            # ring FIFO: feature traffic overlaps the conv blocks and the
            # main loop never waits on a strip load
            prefetched = {ti: load_strip(ti) for ti in range(2)}

            # ---------------- conv blocks (valid convs on padded slice) --
            # conv1: [64,13,21] -> [64,11,19], silu; conv2: -> [C,9,17]
            def rct_block(w1T_s, b1_s, w2T_s, b2_s, cout, out_d):
                c1 = avp.tile([64, 11, 19], F32, tag="av_t")
                for k in range(9):
                    dy, dx = k // 3, k % 3
                    nc.tensor.matmul(
                        out=c1[:],
                        lhsT=w1T_s[:, k, :],
                        rhs=p0[:, dy:dy + 11, dx:dx + 19],
                        start=(k == 0), stop=(k == 8),
                    )
                h1 = singles.tile([64, 11, 19], BF16)
                nc.scalar.activation(out=h1[:], in_=c1[:], func=AF.Silu,
                                     bias=b1_s[:], scale=1.0)
                # zero ring outside the global 17x17 grid (reference zero-pads
                # conv2 input); mask is core-dependent data
                nc.vector.tensor_mul(h1[:], h1[:], hmask_s[:])
                for oc in range(cout // 128 if cout >= 128 else 1):
                    mw = min(128, cout)
                    c2 = avp.tile([mw, 9, 17], F32, tag="av_t")
                    for k in range(9):
                        dy, dx = k // 3, k % 3
                        nc.tensor.matmul(
                            out=c2[:],
                            lhsT=w2T_s[:, k, oc * 128:oc * 128 + mw],
                            rhs=h1[:, dy:dy + 9, dx:dx + 17],
                            start=(k == 0), stop=(k == 8),
                        )
                    o2 = convout.tile([mw, 9, 17], BF16, tag="o2")
                    if mw == 128:
                        bias_ap = b2_s[:, oc:oc + 1]
                    else:
                        bias_ap = b2_s[:]
                    nc.scalar.activation(out=o2[:], in_=c2[:], func=AF.Identity,
                                         bias=bias_ap, scale=1.0)
                    nc.scalar.dma_start(out=out_d[oc * 128:oc * 128 + mw],
                                        in_=o2[:])

            rct_block(w1tT_s, b1t_s, w2tT_s, b2t_s, 48, tl_d)
            rct_block(w1rT_s, b1r_s, w2rT_s, b2r_s, 1024, rl_d)

            # ---------------- gather rk / tv from scratch ----------------
            # Scratch channel order is l-major: rl ch' = l*64 + r (via host
            # weight-col permutation), tl ch' = l*3 + ch.
            # rk_blk[64a+r, k*64 + pair] = r_l[16r+l, ti+di, jjp+8a+dj]
            #   with k = 16*(2di+dj) + l, pair = ti*8 + jjp (tiles j, j+8).
            # tv_blk[64a+k', s*64 + pair] = t_l[ch*16+l, ...], s = 4a+ch;
            #   s = 4a+3 is a ones column (softmax denominator).
            rk_blk = singles.tile([128, 4096], BF16)
            tv_blk = singles.tile([128, 512], BF16)
            nc.vector.memset(tv_blk[:], 0.0)

            rl5 = rl_d[:].rearrange("(l r) y x -> l r y x", l=16, r=64)
            tl5 = tl_d[:].rearrange("(l ch) y x -> l ch y x", l=16, ch=3)
            rkv = rk_blk.rearrange("p (c l pr) -> p c l pr", c=4, l=16, pr=64)
            tvv = tv_blk.rearrange("p (s ti jjp) -> p s ti jjp", s=8, ti=8, jjp=8)
            # tv first (conv-t finishes early), then rk ti-major so
            # strip 0's keys are ready first; alternate rings so neither
            # sequencer serializes the whole gather
            gi = 0
            for a in range(2):
                for di in range(2):
                    for dj in range(2):
                        c4 = 2 * di + dj
                        for ch in range(3):
                            src = tl5[:, ch, di:di + 8,
                                      8 * a + dj:8 * a + dj + 8]
                            setup_engs[gi % 2].dma_start(
                                out=tvv[64 * a + 16 * c4:64 * a + 16 * (c4 + 1),
                                        4 * a + ch, :, :],
                                in_=src)
                            gi += 1
                # ones column (softmax denominator accumulator)
                setup_engs[gi % 2].dma_start(
                    out=tvv[64 * a:64 * (a + 1), 4 * a + 3].rearrange(
                        "p ti jjp -> p (ti jjp)"),
                    in_=ones64[:])
                gi += 1
            for ti in range(8):
                for a in range(2):
                    for di in range(2):
                        for dj in range(2):
                            c4 = 2 * di + dj
                            src = rl5[:, :, ti + di, 8 * a + dj:8 * a + dj + 8]
                            setup_engs[gi % 2].dma_start(
                                out=rkv[64 * a:64 * (a + 1), c4, :,
                                        8 * ti:8 * (ti + 1)],
                                in_=src.transpose([1, 0, 2]))
                            gi += 1

            rkk = rk_blk.rearrange("p (k pr) -> p k pr", k=64, pr=64)
            tvs = tv_blk.rearrange("p (s pr) -> p s pr", s=8, pr=64)

            # ---------------- main loop ----------------
            # KREPEAT>1 re-runs the hot loop in-NEFF (benchmarking only:
            # amplifies kernel time above the per-launch overhead floor).
            repeat = int(os.environ.get("KREPEAT", "1"))
            for ti in [t for _ in range(repeat) for t in range(NROW)]:
                strip = (prefetched.pop(ti) if ti in prefetched
                         else load_strip(ti))

                for g4 in range(2):
                    av_t = avp.tile([128, 1024], F32, tag="av_t")

                    # AV+sum: out rows (4a+s) = (chA0..2,sumA,chB0..2,sumB).
                    # Software pipeline: AV(u-1) is emitted after QK(u) so
                    # the in-order PE stream never stalls waiting for
                    # exp(u-1) — it completes on ACT while QK(u) runs.
                    def emit_av(uu, w_, exp_t, pair):
                        lhsV = tvs[:, :, pair]
                        for v in range(2):
                            po = 32 * (2 * uu + v)
                            nc.tensor.matmul(
                                out=av_t[po:po + 8, 512 * w_:512 * (w_ + 1)],
                                lhsT=lhsV,
                                rhs=exp_t[:, 512 * v:512 * (v + 1)],
                                tile_position=(0, po))

                    pend = None
                    for u in range(4):
                        jjp = 4 * g4 + u
                        pair = ti * 8 + jjp
                        w_, uu = u // 2, u % 2
                        qk_t = qkp.tile([128, 1024], F32, tag="qk_t")
                        lhsA = rkk[0:64, :, pair]
                        lhsB = rkk[64:128, :, pair]
                        for half, lhsH in ((0, lhsA), (64, lhsB)):
                            for v in range(2):
                                rhsH = strip[half:half + 64,
                                             16 * v:16 * (v + 1), jjp, :]
                                nc.tensor.matmul(
                                    out=qk_t[half:half + 64,
                                             512 * v:512 * (v + 1)],
                                    lhsT=lhsH, rhs=rhsH)
                        if pend is not None:
                            emit_av(*pend)
                        exp_t = exps.tile([128, 1024], BF16, tag="exp_t")
                        nc.scalar.activation(out=exp_t[:], in_=qk_t[:],
                                             func=AF.Exp, scale=0.125)
                        pend = (uu, w_, exp_t, pair)
                    emit_av(*pend)

                    # normalize 2 banks (4 pairs x 2 chunks quadrants).
                    # muls scatter into mt2 free=(w,s | 2i+a); the second
                    # 32x32 transpose then lands (v,2i,a) on partitions so
                    # the store balances to a 3-dim DMA per (uu,w):
                    # t2b[64uu + 32v+2i+a, 128w+32s+q] -> yshard[s, y, x]
                    tt = work.tile([128, 1024], F32, tag="tt")
                    nc.vector.transpose(out=tt[:], in_=av_t[:])
                    t5 = tt.rearrange("p (w i r) -> p w i r", w=2, i=16,
                                      r=32)[:, :, :, 0:8].rearrange(
                        "p w i (aa s) -> p w i aa s", aa=2, s=4)
                    rc = work.tile([128, 2, 16, 2], F32, tag="rc")
                    nc.vector.reciprocal(out=rc[:], in_=t5[:, :, :, :, 3])
                    mt2 = work.tile([128, 256], F32, tag="mt")
                    m2v = mt2.rearrange("p (w s iq) -> p w s iq", w=2, s=4,
                                        iq=32).rearrange(
                        "p w s (i aa) -> p w s i aa", i=16, aa=2)
                    for s in range(4):
                        nc.vector.tensor_mul(m2v[:, :, s], t5[:, :, :, :, s],
                                             rc[:])
                    t2b = t2p.tile([128, 256], F32, tag="t2")
                    nc.vector.transpose(out=t2b[:], in_=mt2[:])
                    Yv = yshard[:, 32 * ti:32 * (ti + 1), :].rearrange(
                        "c y x -> c (y x)").rearrange(
                        "c (via rem) -> c via rem", via=64, rem=256)
                    for uu in range(2):
                        for w in range(2):
                            srcap = t2b[64 * uu:64 * (uu + 1),
                                        128 * w:128 * (w + 1)].rearrange(
                                "p (s q) -> p s q", s=4, q=32)[:, 0:3, :]
                            b = 128 * g4 + 64 * w + 32 * uu
                            out_engs = [nc.sync, nc.scalar]
                            out_engs[(w + uu) % 2].dma_start(
                                out=Yv[:, :, b:b + 32].transpose([1, 0, 2]),
                                in_=srcap)
    nc.compile()
    return nc


_PROGRAM_CACHE = {}


def _get_program():
    if "nc" not in _PROGRAM_CACHE:
        _PROGRAM_CACHE["nc"] = build_program()
    return _PROGRAM_CACHE["nc"]


def _prep_inputs(feature, p_low, r_w1, r_b1, r_gamma, r_beta, r_mean, r_var,
                 r_w2, r_b2, t_w1, t_b1, t_gamma, t_beta, t_mean, t_var,
                 t_w2, t_b2):
    f32 = np.float32

    def fold(w1, b1, g, be, m, v):
        s = (g / np.sqrt(v + f32(1e-5))).astype(f32)
        w1f = (w1 * s[:, None, None, None]).astype(f32)
        b1f = ((b1 - m) * s + be).astype(f32)
        return w1f, b1f

    def wT(w):  # [O,C,3,3] -> [C, 9, O]
        return np.ascontiguousarray(w.transpose(1, 2, 3, 0).reshape(
            w.shape[1], 9, w.shape[0])).astype(ml_dtypes.bfloat16)

    w1rf, b1rf = fold(r_w1, r_b1, r_gamma, r_beta, r_mean, r_var)
    w1tf, b1tf = fold(t_w1, t_b1, t_gamma, t_beta, t_mean, t_var)

    # conv2 weight cols permuted l-major: rl col o' = l*64 + r for
    # o = r*16 + l; tl col o' = l*3 + ch for o = ch*16 + l.
    w2r_lm = np.ascontiguousarray(
        r_w2.reshape(64, 16, 64, 3, 3).transpose(2, 3, 4, 1, 0).reshape(
            64, 9, 1024)).astype(f32)
    b2r_lm = np.ascontiguousarray(
        r_b2.reshape(64, 16).T.reshape(1024).reshape(8, 128).T).astype(f32)
    w2t_lm = np.ascontiguousarray(
        t_w2.reshape(3, 16, 64, 3, 3).transpose(2, 3, 4, 1, 0).reshape(
            64, 9, 48)).astype(f32)
    b2t_lm = np.ascontiguousarray(
        t_b2.reshape(3, 16).T.reshape(48, 1)).astype(f32)

    common = {
        "w1rT": wT(w1rf), "b1r": b1rf.reshape(64, 1),
        "w2rT": w2r_lm.astype(ml_dtypes.bfloat16), "b2r": b2r_lm,
        "w1tT": wT(w1tf), "b1t": b1tf.reshape(64, 1),
        "w2tT": w2t_lm.astype(ml_dtypes.bfloat16), "b2t": b2t_lm,
        "ones64": np.ones((64, 64), ml_dtypes.bfloat16),
    }

    # p_low padded: [B, 64, 17, 17] -> per (b,h): [64, 13, 21]
    ppad = np.zeros((p_low.shape[0], 64, 17 + 8, 21), f32)
    ppad[:, :, 2:19, 2:19] = p_low
    in_maps = []
    for core in range(8):
        b, h = core // 2, core % 2
        m = dict(common)
        m["feat"] = np.ascontiguousarray(feature[b, :, 256 * h:256 * (h + 1), :]
                                         ).astype(ml_dtypes.bfloat16)
        m["pshard"] = np.ascontiguousarray(ppad[b, :, 8 * h:8 * h + 13, :]).astype(ml_dtypes.bfloat16)
        mk = np.zeros((11, 19), f32)
        for i in range(11):
            for j in range(19):
                gr, gc = 8 * h - 1 + i, j - 1
                if 0 <= gr <= 16 and 0 <= gc <= 16:
                    mk[i, j] = 1.0
        m["hmask"] = np.ascontiguousarray(
            np.broadcast_to(mk, (64, 11, 19))).astype(ml_dtypes.bfloat16)
        in_maps.append(m)
    return in_maps


def run(inputs, trace=False, tmpdir=None):
    nc = _get_program()
    in_maps = _prep_inputs(**{k: np.asarray(v) for k, v in inputs.items()})
    res = run_bass_kernel_spmd(nc, in_maps, core_ids=list(range(8)),
                               trace=trace, tmpdir=tmpdir)
    Y = np.zeros((4, 3, 512, 512), np.float32)
    for core in range(8):
        b, h = core // 2, core % 2
        Y[b, :, 256 * h:256 * (h + 1), :] = res.results[core]["yshard"]
    return Y, res


def kernel(**inputs):
    return run(inputs)[0]

